# revision 22
# baseline (speedup 1.0000x reference)
"""Trainium2 Bass kernel for nn_CoupledModel: 24 tiny MLPs (2->64->64->1) + free-energy combine.

Strategy (pure data parallel, 8 cores x 16384 rows):
  Pass 1 (per core): for each of 12 network *pairs* (2 nets packed on 128 partitions):
      z1 = W1pair.T @ x            (PE, contraction=2, M=128)
      h1 = tanh(z1 + b1)           (ACT, PSUM->SBUF, per-partition bias)
      z2 = blockdiag(W2).T @ h1    (PE, contraction=128)
      h2 = tanh(z2 + b2)           (ACT)
      sub[2j:2j+2] = W3cols.T @ h2 (PE, M=2, all pairs -> one [24,F] PSUM tile)
    sub tile copied to SBUF (DVE) and DMA'd to DRAM as subT [24, N_core].
  Pass 2 (per core): rows-on-partitions layout [128, g, 24]; softmax-style combine
    using log(w) = e - ln(S) identity; Exp/Ln on ACT, everything else on DVE.

Outputs: subT [24, 16384] per core and outR [128, 128] per core; host reassembles.
"""

import numpy as np

import concourse.bacc as bacc
import concourse.bass as bass
import concourse.tile as tile
from concourse import mybir
from concourse.bass_utils import run_bass_kernel_spmd

NCORES = 8
N = 131072
K = 24
W = 64
NP = K // 2            # 12 pairs
NC_ROWS = N // NCORES  # 16384 rows per core
F = 1024               # batch-tile columns (PSUM: [128,1024] = 2 banks)
FH = 512               # matmul moving-operand chunk (fp32 max)
NT = NC_ROWS // F      # 16 tiles
KB = 0.1

CH = 2048              # pass-2 chunk rows
NCH = NC_ROWS // CH    # 8 chunks
G = CH // 128          # 16 groups per chunk
GC = NC_ROWS // 128    # 128 columns of x1r/outR per core

DT = mybir.dt.float32
DT16 = mybir.dt.float16  # matmul operands: 1 cycle/row, PE runs warm
AF = mybir.ActivationFunctionType
OP = mybir.AluOpType


def _body(ctx, tc, ins, outs):
    nc = tc.nc
    xT, x1r, w1s, b1s, w2s, b2s, w3s = ins
    subT, outR = outs

    const = ctx.enter_context(tc.tile_pool(name="const", bufs=1))
    hpool = ctx.enter_context(tc.tile_pool(name="h", bufs=3))
    spool = ctx.enter_context(tc.tile_pool(name="subs", bufs=2))
    pz1 = ctx.enter_context(tc.tile_pool(name="pz1", bufs=2, space=bass.MemorySpace.PSUM))
    pz2 = ctx.enter_context(tc.tile_pool(name="pz2", bufs=1, space=bass.MemorySpace.PSUM))
    psub = ctx.enter_context(tc.tile_pool(name="psub", bufs=1, space=bass.MemorySpace.PSUM))
    p2 = ctx.enter_context(tc.tile_pool(name="p2", bufs=3))
    p2s = ctx.enter_context(tc.tile_pool(name="p2s", bufs=3))

    # ---- resident constants ----
    w1t = const.tile([2, NP * 128], DT16)
    nc.sync.dma_start(out=w1t, in_=w1s.ap())
    w2t = const.tile([128, NP * 128], DT16)
    nc.sync.dma_start(out=w2t, in_=w2s.ap())
    w3t = const.tile([128, NP * K], DT16)
    nc.sync.dma_start(out=w3t, in_=w3s.ap())
    b1t = const.tile([128, NP], DT)
    nc.sync.dma_start(out=b1t, in_=b1s.ap())
    b2t = const.tile([128, NP], DT)
    nc.sync.dma_start(out=b2t, in_=b2s.ap())
    xt = const.tile([2, NC_ROWS], DT16)
    nc.sync.dma_start(out=xt, in_=xT.ap())

    # ---- pass 2 setup (early: overlaps pass 1 on otherwise-idle DVE) ----
    x1t = const.tile([128, GC], DT)
    nc.sync.dma_start(out=x1t, in_=x1r.ap())
    Tt = const.tile([128, GC], DT)
    nc.vector.tensor_scalar_max(Tt, x1t, 0.1)
    kbTt = const.tile([128, GC], DT)
    nc.vector.tensor_scalar_mul(kbTt, Tt, KB)
    rkTt = const.tile([128, GC], DT)
    nc.vector.reciprocal(rkTt, kbTt)
    # broadcast along a new inner dim of 12
    Trep = const.tile([128, GC, NP], DT)
    nc.vector.tensor_copy(Trep, Tt.unsqueeze(2).to_broadcast([128, GC, NP]))
    rkTrep = const.tile([128, GC, NP], DT)
    nc.vector.tensor_copy(rkTrep, rkTt.unsqueeze(2).to_broadcast([128, GC, NP]))
    # sub in rows-on-partitions layout: s3full[p, gc, kk] = subT[kk, gc*128+p]
    s3full = const.tile([128, GC, K], DT)

    GT = F // 128  # gc columns produced per pass-1 tile
    dma_engines = [nc.sync, nc.gpsimd]

    def mm_fc1(z, j, c0):
        for h in range(2):
            nc.tensor.matmul(
                z[:, h * FH : (h + 1) * FH],
                w1t[:, j * 128 : (j + 1) * 128],
                xt[:, c0 + h * FH : c0 + (h + 1) * FH],
                start=True,
                stop=True,
            )

    # ---- pass 1 (software-pipelined: fc1/tanh1 of pair j+1 overlap fc2/fc3 of j) ----
    for t in range(NT):
        c0 = t * F
        subp = psub.tile([K, F], DT, tag="subp")
        z1 = pz1.tile([128, F], DT, tag="z1")
        mm_fc1(z1, 0, c0)
        h1 = hpool.tile([128, F], DT16, tag="h1")
        nc.scalar.activation(h1, z1, AF.Tanh, bias=b1t[:, 0:1])
        if t == 0:
            # PE warm-up: runs during the first tanh's ACT table load (~2.7us),
            # giving the HAM a fully-busy 3.4us window so the clock gate opens
            # (1.2 -> 2.4 GHz) *after* the one long PE-idle gap of the kernel.
            # Garbage lands in subp and is cleared by fc3 j=0 (start=True).
            for wi in range(18):
                nc.tensor.matmul(
                    subp[:, 0:FH], w3t[:, 0:K], w2t[:, 0:FH], start=True, stop=True
                )
        for j in range(NP):
            h1n = None
            if j + 1 < NP:
                z1n = pz1.tile([128, F], DT, tag="z1")
                mm_fc1(z1n, j + 1, c0)
            # dependency-free PE fillers: keep the HAM activity window dense so
            # the 2.4 GHz clock state survives the short waits on ACT outputs
            for _ in range(3):
                nc.tensor.ldweights(w2t[:, 0:128])
            z2 = pz2.tile([128, F], DT)
            for h in range(2):
                nc.tensor.matmul(
                    z2[:, h * FH : (h + 1) * FH],
                    w2t[:, j * 128 : (j + 1) * 128],
                    h1[:, h * FH : (h + 1) * FH],
                    start=True,
                    stop=True,
                )
            if j + 1 < NP:
                h1n = hpool.tile([128, F], DT16, tag="h1")
                nc.scalar.activation(h1n, z1n, AF.Tanh, bias=b1t[:, j + 1 : j + 2])
            for _ in range(3):
                nc.tensor.ldweights(w2t[:, 0:128])
            h2 = hpool.tile([128, F], DT16, tag="h2")
            nc.scalar.activation(h2, z2, AF.Tanh, bias=b2t[:, j : j + 1])
            for h in range(2):
                nc.tensor.matmul(
                    subp[:, h * FH : (h + 1) * FH],
                    w3t[:, j * K : (j + 1) * K],
                    h2[:, h * FH : (h + 1) * FH],
                    start=(j == 0),
                    stop=(j == NP - 1),
                )
            h1 = h1n
        subsb = spool.tile([K, F], DT)
        nc.vector.tensor_copy(subsb, subp)
        nc.sync.dma_start(out=subT.ap()[:, c0 : c0 + F], in_=subsb)
        # prefetch this tile's rows back in transposed layout (overlaps remaining pass 1)
        for kk in range(K):
            eng = dma_engines[(t * K + kk) % len(dma_engines)]
            eng.dma_start(
                out=s3full[:, t * GT : (t + 1) * GT, kk],
                in_=bass.AP(
                    tensor=subT.ap().tensor,
                    offset=kk * NC_ROWS + c0,
                    ap=[[1, 128], [128, GT]],
                ),
            )

    # ---- pass 2 phase A: everything through Exp, per chunk (ACT set: exp) ----
    SLt = const.tile([128, GC], DT)
    SLvnt = const.tile([128, GC], DT)
    SLet = const.tile([128, GC], DT)
    St = const.tile([128, GC], DT)
    for c in range(NCH):
        g0 = c * G
        s3 = s3full[:, g0 : g0 + G, :]
        sq = p2.tile([128, G, NP], DT, tag="sq")
        nc.vector.tensor_tensor(sq, s3[:, :, 1::2], s3[:, :, 1::2], OP.mult)
        tv = p2.tile([128, G, NP], DT, tag="tv")
        nc.vector.tensor_tensor(tv, sq, Trep[:, g0 : g0 + G, :], OP.mult)
        vneg = p2.tile([128, G, NP], DT, tag="vneg")
        nc.vector.tensor_tensor(vneg, tv, s3[:, :, 0::2], OP.subtract)
        eu = p2.tile([128, G, NP], DT, tag="eu")
        nc.vector.tensor_tensor(eu, vneg, rkTrep[:, g0 : g0 + G, :], OP.mult)
        e = p2.tile([128, G, NP], DT, tag="e")
        nc.vector.tensor_scalar_min(e, eu, 10.0)
        L = p2.tile([128, G, NP], DT, tag="L")
        nc.scalar.activation(L, e, AF.Exp)
        nc.vector.tensor_reduce(SLt[:, g0 : g0 + G], L, mybir.AxisListType.X, OP.add)
        Lv = p2.tile([128, G, NP], DT, tag="Lv")
        nc.vector.tensor_tensor(Lv, L, vneg, OP.mult)
        nc.vector.tensor_reduce(SLvnt[:, g0 : g0 + G], Lv, mybir.AxisListType.X, OP.add)
        Le = p2.tile([128, G, NP], DT, tag="Le")
        nc.vector.tensor_tensor(Le, L, e, OP.mult)
        nc.vector.tensor_reduce(SLet[:, g0 : g0 + G], Le, mybir.AxisListType.X, OP.add)
        nc.vector.tensor_scalar_add(St[:, g0 : g0 + G], SLt[:, g0 : g0 + G], 1e-9)

    # ---- pass 2 phase B: Ln + final combine, whole width at once ----
    lnS = const.tile([128, GC], DT)
    nc.scalar.activation(lnS, St, AF.Ln)
    rS = const.tile([128, GC], DT)
    nc.vector.reciprocal(rS, St)
    m1 = const.tile([128, GC], DT)
    nc.vector.tensor_tensor(m1, lnS, SLt, OP.mult)
    m2 = const.tile([128, GC], DT)
    nc.vector.tensor_tensor(m2, SLet, m1, OP.subtract)
    m3 = const.tile([128, GC], DT)
    nc.vector.tensor_tensor(m3, m2, kbTt, OP.mult)
    m4 = const.tile([128, GC], DT)
    nc.vector.tensor_tensor(m4, m3, SLvnt, OP.subtract)
    ov = const.tile([128, GC], DT)
    nc.vector.tensor_tensor(ov, m4, rS, OP.mult)
    nc.sync.dma_start(out=outR.ap(), in_=ov)


_CACHE = {}


def _build():
    if "nc" in _CACHE:
        return _CACHE["nc"]
    nc = bacc.Bacc("TRN2", target_bir_lowering=False, debug=False)
    ins = [
        nc.dram_tensor("xT", [2, NC_ROWS], DT16, kind="ExternalInput"),
        nc.dram_tensor("x1r", [128, GC], DT, kind="ExternalInput"),
        nc.dram_tensor("w1s", [2, NP * 128], DT16, kind="ExternalInput"),
        nc.dram_tensor("b1s", [128, NP], DT, kind="ExternalInput"),
        nc.dram_tensor("w2s", [128, NP * 128], DT16, kind="ExternalInput"),
        nc.dram_tensor("b2s", [128, NP], DT, kind="ExternalInput"),
        nc.dram_tensor("w3s", [128, NP * K], DT16, kind="ExternalInput"),
    ]
    outs = [
        nc.dram_tensor("subT", [K, NC_ROWS], DT, kind="ExternalOutput"),
        nc.dram_tensor("outR", [128, GC], DT, kind="ExternalOutput"),
    ]
    from contextlib import ExitStack

    with tile.TileContext(nc) as tc, ExitStack() as ctx:
        _body(ctx, tc, ins, outs)
    nc.compile()
    _CACHE["nc"] = nc
    return nc


def _pack_weights(W1, b1, W2, b2, W3):
    w1s = np.zeros((2, NP * 128), np.float32)
    b1s = np.zeros((128, NP), np.float32)
    w2s = np.zeros((128, NP * 128), np.float32)
    b2s = np.zeros((128, NP), np.float32)
    w3s = np.zeros((128, NP * K), np.float32)
    for j in range(NP):
        k0, k1 = 2 * j, 2 * j + 1
        w1s[:, j * 128 : j * 128 + 64] = W1[k0].T
        w1s[:, j * 128 + 64 : j * 128 + 128] = W1[k1].T
        b1s[0:64, j] = b1[k0]
        b1s[64:128, j] = b1[k1]
        w2s[0:64, j * 128 : j * 128 + 64] = W2[k0].T
        w2s[64:128, j * 128 + 64 : j * 128 + 128] = W2[k1].T
        b2s[0:64, j] = b2[k0]
        b2s[64:128, j] = b2[k1]
        w3s[0:64, j * K + 2 * j] = W3[k0]
        w3s[64:128, j * K + 2 * j + 1] = W3[k1]
    return (
        w1s.astype(np.float16),
        b1s,
        w2s.astype(np.float16),
        b2s,
        w3s.astype(np.float16),
    )


TRACE = False
LAST_RESULTS = None


def kernel(x, W1, b1, W2, b2, W3):
    global LAST_RESULTS
    x = np.asarray(x, np.float32)
    w1s, b1s, w2s, b2s, w3s = _pack_weights(
        np.asarray(W1, np.float32),
        np.asarray(b1, np.float32),
        np.asarray(W2, np.float32),
        np.asarray(b2, np.float32),
        np.asarray(W3, np.float32),
    )
    nc = _build()
    in_maps = []
    for c in range(NCORES):
        xs = x[c * NC_ROWS : (c + 1) * NC_ROWS]
        in_maps.append(
            {
                "xT": np.ascontiguousarray(xs.T.astype(np.float16)),
                "x1r": np.ascontiguousarray(xs[:, 1].reshape(GC, 128).T),
                "w1s": w1s,
                "b1s": b1s,
                "w2s": w2s,
                "b2s": b2s,
                "w3s": w3s,
            }
        )
    res = run_bass_kernel_spmd(nc, in_maps, core_ids=list(range(NCORES)), trace=TRACE)
    LAST_RESULTS = res
    sub = np.concatenate([r["subT"].T for r in res.results], axis=0)
    out = np.concatenate([r["outR"].T.reshape(-1) for r in res.results])[:, None]
    return out.astype(np.float32), sub.astype(np.float32)


# revision 24
# speedup vs baseline: 1.6680x; 1.6680x over previous
"""Trainium2 Bass kernel for nn_CoupledModel: 24 tiny MLPs (2->64->64->1) + free-energy combine.

Strategy (pure data parallel, 8 cores x 16384 rows):
  Pass 1 (per core): for each of 12 network *pairs* (2 nets packed on 128 partitions):
      z1 = W1pair.T @ x            (PE, contraction=2, M=128)
      h1 = tanh(z1 + b1)           (ACT, PSUM->SBUF, per-partition bias)
      z2 = blockdiag(W2).T @ h1    (PE, contraction=128)
      h2 = tanh(z2 + b2)           (ACT)
      sub[2j:2j+2] = W3cols.T @ h2 (PE, M=2, all pairs -> one [24,F] PSUM tile)
    sub tile copied to SBUF (DVE) and DMA'd to DRAM as subT [24, N_core].
  Pass 2 (per core): rows-on-partitions layout [128, g, 24]; softmax-style combine
    using log(w) = e - ln(S) identity; Exp/Ln on ACT, everything else on DVE.

Outputs: subT [24, 16384] per core and outR [128, 128] per core; host reassembles.
"""

import numpy as np

import concourse.bacc as bacc
import concourse.bass as bass
import concourse.tile as tile
from concourse import mybir
from concourse.bass_utils import run_bass_kernel_spmd

NCORES = 8
N = 131072
K = 24
W = 64
NP = K // 2            # 12 pairs
NC_ROWS = N // NCORES  # 16384 rows per core
F = 1024               # batch-tile columns (PSUM: [128,1024] = 2 banks)
FH = 512               # matmul moving-operand chunk (fp32 max)
NT = NC_ROWS // F      # 16 tiles
KB = 0.1

CH = 2048              # pass-2 chunk rows
NCH = NC_ROWS // CH    # 8 chunks
G = CH // 128          # 16 groups per chunk
GC = NC_ROWS // 128    # 128 columns of x1r/outR per core

DT = mybir.dt.float32
DT16 = mybir.dt.float16  # matmul operands: 1 cycle/row, PE runs warm
AF = mybir.ActivationFunctionType
OP = mybir.AluOpType


def _body(ctx, tc, ins, outs):
    nc = tc.nc
    xT, x1r, w1s, b1s, w2s, b2s, w3s = ins
    subT, outR = outs

    const = ctx.enter_context(tc.tile_pool(name="const", bufs=1))
    hpool = ctx.enter_context(tc.tile_pool(name="h", bufs=3))
    spool = ctx.enter_context(tc.tile_pool(name="subs", bufs=2))
    pzz = ctx.enter_context(tc.tile_pool(name="pzz", bufs=3, space=bass.MemorySpace.PSUM))
    psub = ctx.enter_context(tc.tile_pool(name="psub", bufs=1, space=bass.MemorySpace.PSUM))
    p2 = ctx.enter_context(tc.tile_pool(name="p2", bufs=3))
    p2s = ctx.enter_context(tc.tile_pool(name="p2s", bufs=3))

    # ---- resident constants ----
    w1t = const.tile([2, NP * 128], DT16)
    nc.sync.dma_start(out=w1t, in_=w1s.ap())
    w2t = const.tile([128, NP * 128], DT16)
    nc.sync.dma_start(out=w2t, in_=w2s.ap())
    w3t = const.tile([128, NP * K], DT16)
    nc.sync.dma_start(out=w3t, in_=w3s.ap())
    b1t = const.tile([128, NP], DT)
    nc.sync.dma_start(out=b1t, in_=b1s.ap())
    b2t = const.tile([128, NP], DT)
    nc.sync.dma_start(out=b2t, in_=b2s.ap())
    xt = const.tile([2, NC_ROWS], DT16)
    nc.sync.dma_start(out=xt, in_=xT.ap())

    # ---- pass 2 setup (early: overlaps pass 1 on otherwise-idle DVE) ----
    x1t = const.tile([128, GC], DT)
    nc.sync.dma_start(out=x1t, in_=x1r.ap())
    Tt = const.tile([128, GC], DT)
    nc.vector.tensor_scalar_max(Tt, x1t, 0.1)
    kbTt = const.tile([128, GC], DT)
    nc.vector.tensor_scalar_mul(kbTt, Tt, KB)
    rkTt = const.tile([128, GC], DT)
    nc.vector.reciprocal(rkTt, kbTt)
    # broadcast along a new inner dim of 12
    Trep = const.tile([128, GC, NP], DT)
    nc.vector.tensor_copy(Trep, Tt.unsqueeze(2).to_broadcast([128, GC, NP]))
    rkTrep = const.tile([128, GC, NP], DT)
    nc.vector.tensor_copy(rkTrep, rkTt.unsqueeze(2).to_broadcast([128, GC, NP]))
    # sub in rows-on-partitions layout: s3full[p, gc, kk] = subT[kk, gc*128+p]
    s3full = const.tile([128, GC, K], DT)

    GT = F // 128  # gc columns produced per pass-1 tile
    dma_engines = [nc.sync, nc.gpsimd]

    def mm_fc1(z, j, c0):
        for h in range(2):
            nc.tensor.matmul(
                z[:, h * FH : (h + 1) * FH],
                w1t[:, j * 128 : (j + 1) * 128],
                xt[:, c0 + h * FH : c0 + (h + 1) * FH],
                start=True,
                stop=True,
            )

    # ---- pass 1 (software-pipelined: fc1/tanh1 of pair j+1 overlap fc2/fc3 of j) ----
    def mm_fc3(subp, j, h2j):
        for h in range(2):
            nc.tensor.matmul(
                subp[:, h * FH : (h + 1) * FH],
                w3t[:, j * K : (j + 1) * K],
                h2j[:, h * FH : (h + 1) * FH],
                start=(j == 0),
                stop=(j == NP - 1),
            )

    for t in range(NT):
        c0 = t * F
        subp = psub.tile([K, F], DT, tag="subp")
        z1 = pzz.tile([128, F], DT, tag="z")
        mm_fc1(z1, 0, c0)
        h1 = hpool.tile([128, F], DT16, tag="h1")
        nc.scalar.activation(h1, z1, AF.Tanh, bias=b1t[:, 0:1])
        if t == 0:
            # PE warm-up: runs during the first tanh's ACT table load (~2.7us),
            # giving the HAM a fully-busy 3.4us window so the clock gate opens
            # (1.2 -> 2.4 GHz) *after* the one long PE-idle gap of the kernel.
            # Garbage lands in subp and is cleared by fc3 j=0 (start=True).
            for wi in range(18):
                nc.tensor.matmul(
                    subp[:, 0:FH], w3t[:, 0:K], w2t[:, 0:FH], start=True, stop=True
                )
        h2prev = None
        for j in range(NP):
            h1n = None
            if j + 1 < NP:
                z1n = pzz.tile([128, F], DT, tag="z")
                mm_fc1(z1n, j + 1, c0)
            z2 = pzz.tile([128, F], DT, tag="z")
            for h in range(2):
                nc.tensor.matmul(
                    z2[:, h * FH : (h + 1) * FH],
                    w2t[:, j * 128 : (j + 1) * 128],
                    h1[:, h * FH : (h + 1) * FH],
                    start=True,
                    stop=True,
                )
            if h2prev is not None:
                mm_fc3(subp, j - 1, h2prev)
            if j + 1 < NP:
                h1n = hpool.tile([128, F], DT16, tag="h1")
                nc.scalar.activation(h1n, z1n, AF.Tanh, bias=b1t[:, j + 1 : j + 2])
            h2 = hpool.tile([128, F], DT16, tag="h2")
            nc.scalar.activation(h2, z2, AF.Tanh, bias=b2t[:, j : j + 1])
            h2prev = h2
            h1 = h1n
        mm_fc3(subp, NP - 1, h2prev)
        subsb = spool.tile([K, F], DT)
        nc.vector.tensor_copy(subsb, subp)
        nc.sync.dma_start(out=subT.ap()[:, c0 : c0 + F], in_=subsb)
        # prefetch this tile's rows back in transposed layout (overlaps remaining pass 1)
        for kk in range(K):
            eng = dma_engines[(t * K + kk) % len(dma_engines)]
            eng.dma_start(
                out=s3full[:, t * GT : (t + 1) * GT, kk],
                in_=bass.AP(
                    tensor=subT.ap().tensor,
                    offset=kk * NC_ROWS + c0,
                    ap=[[1, 128], [128, GT]],
                ),
            )

    # ---- pass 2 phase A: everything through Exp, per chunk (ACT set: exp) ----
    SLt = const.tile([128, GC], DT)
    SLvnt = const.tile([128, GC], DT)
    SLet = const.tile([128, GC], DT)
    St = const.tile([128, GC], DT)
    for c in range(NCH):
        g0 = c * G
        s3 = s3full[:, g0 : g0 + G, :]
        sq = p2.tile([128, G, NP], DT, tag="sq")
        nc.vector.tensor_tensor(sq, s3[:, :, 1::2], s3[:, :, 1::2], OP.mult)
        tv = p2.tile([128, G, NP], DT, tag="tv")
        nc.vector.tensor_tensor(tv, sq, Trep[:, g0 : g0 + G, :], OP.mult)
        vneg = p2.tile([128, G, NP], DT, tag="vneg")
        nc.vector.tensor_tensor(vneg, tv, s3[:, :, 0::2], OP.subtract)
        eu = p2.tile([128, G, NP], DT, tag="eu")
        nc.vector.tensor_tensor(eu, vneg, rkTrep[:, g0 : g0 + G, :], OP.mult)
        e = p2.tile([128, G, NP], DT, tag="e")
        nc.vector.tensor_scalar_min(e, eu, 10.0)
        L = p2.tile([128, G, NP], DT, tag="L")
        nc.scalar.activation(L, e, AF.Exp)
        nc.vector.tensor_reduce(SLt[:, g0 : g0 + G], L, mybir.AxisListType.X, OP.add)
        Lv = p2.tile([128, G, NP], DT, tag="Lv")
        nc.vector.tensor_tensor(Lv, L, vneg, OP.mult)
        nc.vector.tensor_reduce(SLvnt[:, g0 : g0 + G], Lv, mybir.AxisListType.X, OP.add)
        Le = p2.tile([128, G, NP], DT, tag="Le")
        nc.vector.tensor_tensor(Le, L, e, OP.mult)
        nc.vector.tensor_reduce(SLet[:, g0 : g0 + G], Le, mybir.AxisListType.X, OP.add)
        nc.vector.tensor_scalar_add(St[:, g0 : g0 + G], SLt[:, g0 : g0 + G], 1e-9)

    # ---- pass 2 phase B: Ln + final combine, whole width at once ----
    lnS = const.tile([128, GC], DT)
    nc.scalar.activation(lnS, St, AF.Ln)
    rS = const.tile([128, GC], DT)
    nc.vector.reciprocal(rS, St)
    m1 = const.tile([128, GC], DT)
    nc.vector.tensor_tensor(m1, lnS, SLt, OP.mult)
    m2 = const.tile([128, GC], DT)
    nc.vector.tensor_tensor(m2, SLet, m1, OP.subtract)
    m3 = const.tile([128, GC], DT)
    nc.vector.tensor_tensor(m3, m2, kbTt, OP.mult)
    m4 = const.tile([128, GC], DT)
    nc.vector.tensor_tensor(m4, m3, SLvnt, OP.subtract)
    ov = const.tile([128, GC], DT)
    nc.vector.tensor_tensor(ov, m4, rS, OP.mult)
    nc.sync.dma_start(out=outR.ap(), in_=ov)


_CACHE = {}


def _build():
    if "nc" in _CACHE:
        return _CACHE["nc"]
    nc = bacc.Bacc("TRN2", target_bir_lowering=False, debug=False)
    ins = [
        nc.dram_tensor("xT", [2, NC_ROWS], DT16, kind="ExternalInput"),
        nc.dram_tensor("x1r", [128, GC], DT, kind="ExternalInput"),
        nc.dram_tensor("w1s", [2, NP * 128], DT16, kind="ExternalInput"),
        nc.dram_tensor("b1s", [128, NP], DT, kind="ExternalInput"),
        nc.dram_tensor("w2s", [128, NP * 128], DT16, kind="ExternalInput"),
        nc.dram_tensor("b2s", [128, NP], DT, kind="ExternalInput"),
        nc.dram_tensor("w3s", [128, NP * K], DT16, kind="ExternalInput"),
    ]
    outs = [
        nc.dram_tensor("subT", [K, NC_ROWS], DT, kind="ExternalOutput"),
        nc.dram_tensor("outR", [128, GC], DT, kind="ExternalOutput"),
    ]
    from contextlib import ExitStack

    with tile.TileContext(nc) as tc, ExitStack() as ctx:
        _body(ctx, tc, ins, outs)
    nc.compile()
    _CACHE["nc"] = nc
    return nc


def _pack_weights(W1, b1, W2, b2, W3):
    w1s = np.zeros((2, NP * 128), np.float32)
    b1s = np.zeros((128, NP), np.float32)
    w2s = np.zeros((128, NP * 128), np.float32)
    b2s = np.zeros((128, NP), np.float32)
    w3s = np.zeros((128, NP * K), np.float32)
    for j in range(NP):
        k0, k1 = 2 * j, 2 * j + 1
        w1s[:, j * 128 : j * 128 + 64] = W1[k0].T
        w1s[:, j * 128 + 64 : j * 128 + 128] = W1[k1].T
        b1s[0:64, j] = b1[k0]
        b1s[64:128, j] = b1[k1]
        w2s[0:64, j * 128 : j * 128 + 64] = W2[k0].T
        w2s[64:128, j * 128 + 64 : j * 128 + 128] = W2[k1].T
        b2s[0:64, j] = b2[k0]
        b2s[64:128, j] = b2[k1]
        w3s[0:64, j * K + 2 * j] = W3[k0]
        w3s[64:128, j * K + 2 * j + 1] = W3[k1]
    return (
        w1s.astype(np.float16),
        b1s,
        w2s.astype(np.float16),
        b2s,
        w3s.astype(np.float16),
    )


TRACE = False
LAST_RESULTS = None


def kernel(x, W1, b1, W2, b2, W3):
    global LAST_RESULTS
    x = np.asarray(x, np.float32)
    w1s, b1s, w2s, b2s, w3s = _pack_weights(
        np.asarray(W1, np.float32),
        np.asarray(b1, np.float32),
        np.asarray(W2, np.float32),
        np.asarray(b2, np.float32),
        np.asarray(W3, np.float32),
    )
    nc = _build()
    in_maps = []
    for c in range(NCORES):
        xs = x[c * NC_ROWS : (c + 1) * NC_ROWS]
        in_maps.append(
            {
                "xT": np.ascontiguousarray(xs.T.astype(np.float16)),
                "x1r": np.ascontiguousarray(xs[:, 1].reshape(GC, 128).T),
                "w1s": w1s,
                "b1s": b1s,
                "w2s": w2s,
                "b2s": b2s,
                "w3s": w3s,
            }
        )
    res = run_bass_kernel_spmd(nc, in_maps, core_ids=list(range(NCORES)), trace=TRACE)
    LAST_RESULTS = res
    sub = np.concatenate([r["subT"].T for r in res.results], axis=0)
    out = np.concatenate([r["outR"].T.reshape(-1) for r in res.results])[:, None]
    return out.astype(np.float32), sub.astype(np.float32)
